# revision 1
# baseline (speedup 1.0000x reference)
"""CGCNNConv on 8 TRN2 NeuronCores.

Strategy (atom-sharded, 12500 atoms/core padded to 12800):
  Host (sharding/layout prep): gather atom_fea[nbr_idx], transpose per-512-atom
  tiles to feature-major bf16, pack [nbrT; gathT] as the K=128 moving operand.
  Device per core, single NEFF:
    pass1: per (t, m): matmul y = W_en^T @ mv + W_self^T @ atomT -> PSUM [128, 512]
           ACT copies y -> bf16 SBUF (+ accumulates sum(y)), DVE accumulates sum(y^2),
           y stored to DRAM scratch [128chans, 153600 edges] bf16.
    AllReduce [128, 2] -> BN1 scale/shift (linear bias b folds out entirely).
    pass2: reload y halves stacked to full 128 partitions, ACT applies
           sigmoid(s*f+t) and softplus(s*c+t), DVE mul+accumulate -> msg [64, 12800].
    AllReduce [64, 2] -> BN2 scale/shift.
    pass3: softplus(bn2(msg)), PE-transpose back to row-major, add atom_fea, DMA out.
"""
import os
import sys
sys.path.insert(0, '/opt/trn_rl_repo')
import numpy as np
import ml_dtypes

from concourse import bass, mybir
from concourse.tile import TileContext
from concourse.bass_utils import run_bass_kernel_spmd

NCORES = 8
N = 100000
M = 12
F = 64                      # atom/nbr feature len
OUT = 128                   # 2F
NLOC = 12500                # atoms per core
NPAD = 12800                # padded atoms per core
NT = 25                     # tiles of 512 atoms
TW = 512                    # tile width (atoms)
NQ = NT * M                 # 300 (t, m) chunks
E = NQ * TW                 # 153600 edge slots per core
EPS = 1e-5
BN1_CNT = float(N * M)
BN2_CNT = float(N)

AF = mybir.ActivationFunctionType
f32 = mybir.dt.float32
bf16 = mybir.dt.bfloat16


def _split_wait_lists(nc, limit=1):
    """This walrus codegen accepts at most one sync wait per instruction on
    several ISA structs; move excess waits onto preceding same-engine NoOp
    carriers (sequential same-engine execution keeps the conjunction)."""
    for bbname, bbw in nc.bb_map.items():
        il = bbw.bb.instructions
        i = 0
        while i < len(il):
            inst = il[i]
            if inst.engine in (mybir.EngineType.Pool, mybir.EngineType.PE) or \
                    type(inst).__name__ in ('InstISA',):
                i += 1
                continue
            si = getattr(inst, 'sync_info', None)
            waits = list(si.on_wait) if si is not None and si.on_wait is not None else []
            if len(waits) > limit:
                extra, keep = waits[:-limit], waits[-limit:]
                pos = i
                # don't break fused pairs (LdWeights+Matmult): insert before
                # the paired loader
                while pos > 0 and type(il[pos - 1]).__name__ in (
                        'InstLdweights', 'InstTensorLoad'):
                    pos -= 1
                for j in range(0, len(extra), limit):
                    nd = mybir.InstDrain(
                        name=nc.get_next_instruction_name(), ins=[], outs=[])
                    nd.engine = inst.engine
                    nd.sync_info = mybir.SyncInfo(
                        on_wait=list(extra[j:j + limit]), on_update=[])
                    il.insert(pos, nd)
                    pos += 1
                    i += 1
                inst.sync_info = mybir.SyncInfo(
                    on_wait=list(keep), on_update=list(si.on_update))
            i += 1


def _build():
    PH = os.environ.get("KPHASES", "123")
    nc = bass.Bass(num_devices=NCORES)

    mvT = nc.declare_dram_parameter("mvT", [NT, M, 128, TW], bf16, isOutput=False)
    atomT = nc.declare_dram_parameter("atomT", [NT, 64, TW], bf16, isOutput=False)
    aself = nc.declare_dram_parameter("aself", [NPAD, F], f32, isOutput=False)
    w_en = nc.declare_dram_parameter("w_en", [128, OUT], f32, isOutput=False)
    w_self = nc.declare_dram_parameter("w_self", [64, OUT], f32, isOutput=False)
    gam1 = nc.declare_dram_parameter("gam1", [OUT, 1], f32, isOutput=False)
    bet1 = nc.declare_dram_parameter("bet1", [OUT, 1], f32, isOutput=False)
    gam2 = nc.declare_dram_parameter("gam2", [F, 1], f32, isOutput=False)
    bet2 = nc.declare_dram_parameter("bet2", [F, 1], f32, isOutput=False)
    ident = nc.declare_dram_parameter("ident", [64, 64], f32, isOutput=False)
    out = nc.declare_dram_parameter("out", [NPAD, F], f32, isOutput=True)

    y_dram = nc.dram_tensor("y_dram", [128, E], bf16)
    st1_loc = nc.dram_tensor("st1_loc", [OUT, 2], f32)
    st1_sh = nc.dram_tensor("st1_sh", [OUT, 2], f32, addr_space="Shared")
    st2_loc = nc.dram_tensor("st2_loc", [F, 2], f32)
    st2_sh = nc.dram_tensor("st2_sh", [F, 2], f32, addr_space="Shared")

    groups = [list(range(NCORES))]

    with TileContext(nc, num_cores=NCORES) as tc:
        with tc.tile_pool(name="const", bufs=1) as cpool, \
             tc.tile_pool(name="work", bufs=4) as pool, \
             tc.tile_pool(name="acc", bufs=1) as apool, \
             tc.tile_pool(name="psum", bufs=3, space="PSUM") as pp:

            # --- constants ---
            w_en_f = cpool.tile([128, OUT], f32)
            nc.sync.dma_start(out=w_en_f[:], in_=w_en[:])
            w_en_b = cpool.tile([128, OUT], bf16)
            nc.vector.tensor_copy(out=w_en_b[:], in_=w_en_f[:])
            w_self_f = cpool.tile([64, OUT], f32)
            nc.sync.dma_start(out=w_self_f[:], in_=w_self[:])
            w_self_b = cpool.tile([64, OUT], bf16)
            nc.vector.tensor_copy(out=w_self_b[:], in_=w_self_f[:])
            id_raw = cpool.tile([64, 64], f32)
            nc.sync.dma_start(out=id_raw[:], in_=ident[:])
            id_sb = cpool.tile([64, 64], f32)
            nc.vector.tensor_copy(out=id_sb[:], in_=id_raw[:])
            g1_sb = cpool.tile([OUT, 1], f32)
            nc.sync.dma_start(out=g1_sb[:], in_=gam1[:])
            b1_sb = cpool.tile([OUT, 1], f32)
            nc.sync.dma_start(out=b1_sb[:], in_=bet1[:])
            g2_sb = cpool.tile([F, 1], f32)
            nc.sync.dma_start(out=g2_sb[:], in_=gam2[:])
            b2_sb = cpool.tile([F, 1], f32)
            nc.sync.dma_start(out=b2_sb[:], in_=bet2[:])

            ysum = apool.tile([128, NQ], f32)
            ysq = apool.tile([128, NQ], f32)
            msg = apool.tile([64, NPAD], f32)

            # --- pass 1 ---
            for t in range(int(os.environ.get("KNT1", NT))):
                at_raw = pool.tile([64, TW], bf16, tag="atraw")
                nc.sync.dma_start(out=at_raw[:], in_=atomT[t])
                at = pool.tile([64, TW], bf16, tag="at")
                nc.vector.tensor_copy(out=at[:], in_=at_raw[:])
                for m in range(M):
                    q = t * M + m
                    mv_raw = pool.tile([128, TW], bf16, tag="mvraw")
                    nc.sync.dma_start(out=mv_raw[:], in_=mvT[t, m])
                    # DVE relay so every PE input dep is a single DVE semaphore
                    mv = pool.tile([128, TW], bf16, tag="mv")
                    nc.vector.tensor_copy(out=mv[:], in_=mv_raw[:])
                    yp = pp.tile([128, TW], f32, tag="yp")
                    nc.tensor.matmul(yp[:], lhsT=w_en_b[:], rhs=mv[:], start=True, stop=False)
                    nc.tensor.matmul(yp[:], lhsT=w_self_b[:], rhs=at[:], start=False, stop=True)
                    # DVE: psum -> bf16 sbuf copy, fused sum(y) accumulation
                    y_sb = pool.tile([128, TW], bf16, tag="ysb")
                    nc.vector.tensor_scalar(out=y_sb[:], in0=yp[:], scalar1=1.0,
                                            scalar2=0.0, op0=mybir.AluOpType.mult,
                                            op1=mybir.AluOpType.add,
                                            accum_out=ysum[:, q:q + 1])
                    # ACT: sum(y^2) from the bf16 copy
                    sqt = pool.tile([128, TW], f32, tag="sqt")
                    nc.scalar.activation(out=sqt[:], in_=y_sb[:], func=AF.Square,
                                         accum_out=ysq[:, q:q + 1])
                    nc.scalar.dma_start(out=y_dram[:, q * TW:(q + 1) * TW], in_=y_sb[:])

            # --- BN1 stats: reduce + allreduce ---
            st1 = apool.tile([OUT, 2], f32)
            nc.vector.tensor_reduce(st1[:, 0:1], ysum[:], axis=mybir.AxisListType.X,
                                    op=mybir.AluOpType.add)
            nc.vector.tensor_reduce(st1[:, 1:2], ysq[:], axis=mybir.AxisListType.X,
                                    op=mybir.AluOpType.add)
            nc.sync.dma_start(out=st1_loc[:], in_=st1[:])
            nc.gpsimd.collective_compute(
                "AllReduce", mybir.AluOpType.add, replica_groups=groups,
                ins=[st1_loc[:]], outs=[st1_sh[:]])
            st1g = apool.tile([OUT, 2], f32)
            nc.sync.dma_start(out=st1g[:], in_=st1_sh[:])

            # s1 = gam1 / sqrt(var + eps); t1 = bet1 - mean * s1
            mu1 = apool.tile([OUT, 1], f32)
            nc.vector.tensor_scalar_mul(mu1[:], st1g[:, 0:1], 1.0 / BN1_CNT)
            var1 = apool.tile([OUT, 1], f32)
            nc.vector.tensor_scalar_mul(var1[:], st1g[:, 1:2], 1.0 / BN1_CNT)
            musq = apool.tile([OUT, 1], f32)
            nc.vector.tensor_mul(musq[:], mu1[:], mu1[:])
            nc.vector.tensor_sub(var1[:], var1[:], musq[:])
            nc.vector.tensor_scalar_add(var1[:], var1[:], EPS)
            lnv1 = apool.tile([OUT, 1], f32)
            nc.scalar.activation(out=lnv1[:], in_=var1[:], func=AF.Ln)
            nc.vector.tensor_scalar_mul(lnv1[:], lnv1[:], -0.5)
            inv1 = apool.tile([OUT, 1], f32)
            nc.scalar.activation(out=inv1[:], in_=lnv1[:], func=AF.Exp)
            s1 = apool.tile([OUT, 1], f32)
            nc.vector.tensor_mul(s1[:], g1_sb[:], inv1[:])
            t1 = apool.tile([OUT, 1], f32)
            nc.vector.tensor_mul(t1[:], mu1[:], s1[:])
            nc.vector.tensor_sub(t1[:], b1_sb[:], t1[:])

            # stacked scale/shift: [s_f; s_f], [t_f; t_f], [s_c; s_c], [t_c; t_c]
            sf2 = apool.tile([128, 1], f32)
            tf2 = apool.tile([128, 1], f32)
            sc2 = apool.tile([128, 1], f32)
            tc2 = apool.tile([128, 1], f32)
            nc.vector.tensor_copy(out=sf2[0:64, :], in_=s1[0:64, :])
            nc.vector.tensor_copy(out=tf2[0:64, :], in_=t1[0:64, :])
            nc.sync.dma_start(out=sf2[64:128, :], in_=s1[0:64, :])
            nc.sync.dma_start(out=tf2[64:128, :], in_=t1[0:64, :])
            nc.sync.dma_start(out=sc2[0:64, :], in_=s1[64:128, :])
            nc.sync.dma_start(out=tc2[0:64, :], in_=t1[64:128, :])
            nc.sync.dma_start(out=sc2[64:128, :], in_=s1[64:128, :])
            nc.sync.dma_start(out=tc2[64:128, :], in_=t1[64:128, :])
            sfn = apool.tile([128, 1], f32)
            tfn = apool.tile([128, 1], f32)
            nc.vector.tensor_scalar_mul(sfn[:], sf2[:], -1.0)
            nc.vector.tensor_scalar_mul(tfn[:], tf2[:], -1.0)

            # --- pass 2: msg = sum_m sigmoid(f)*softplus(c) ---
            for t in range(NT if "2" in PH else 0):
                macc = pool.tile([128, TW], f32, tag="macc")
                for k in range(M // 2):
                    q0 = (t * M + 2 * k) * TW
                    q1 = (t * M + 2 * k + 1) * TW
                    yf2 = pool.tile([128, TW], bf16, tag="yf2")
                    nc.sync.dma_start(out=yf2[0:64, :], in_=y_dram[0:64, q0:q0 + TW])
                    nc.sync.dma_start(out=yf2[64:128, :], in_=y_dram[0:64, q1:q1 + TW])
                    yc2 = pool.tile([128, TW], bf16, tag="yc2")
                    nc.scalar.dma_start(out=yc2[0:64, :], in_=y_dram[64:128, q0:q0 + TW])
                    nc.scalar.dma_start(out=yc2[64:128, :], in_=y_dram[64:128, q1:q1 + TW])
                    # sigmoid(x_f) = 1/(1 + exp(-x_f)); softplus(x_c) = ln(exp(x_c)+1)
                    ef = pool.tile([128, TW], f32, tag="ef")
                    nc.scalar.activation(out=ef[:], in_=yf2[:], func=AF.Exp,
                                         bias=tfn[:, 0:1], scale=sfn[:, 0:1])
                    ec = pool.tile([128, TW], f32, tag="ec")
                    nc.scalar.activation(out=ec[:], in_=yc2[:], func=AF.Exp,
                                         bias=tc2[:, 0:1], scale=sc2[:, 0:1])
                    spl = pool.tile([128, TW], f32, tag="spl")
                    nc.scalar.activation(out=spl[:], in_=ec[:], func=AF.Ln, bias=1.0)
                    nc.vector.tensor_scalar_add(ef[:], ef[:], 1.0)
                    rec = pool.tile([128, TW], f32, tag="rec")
                    nc.vector.reciprocal(rec[:], ef[:])
                    if k == 0:
                        nc.vector.tensor_mul(macc[:], spl[:], rec[:])
                    else:
                        prod = pool.tile([128, TW], f32, tag="prod")
                        nc.vector.tensor_mul(prod[:], spl[:], rec[:])
                        nc.vector.tensor_add(macc[:], macc[:], prod[:])
                # fold top (m even) + bottom (m odd): shift bottom to partitions 0-63
                mlo = pool.tile([64, TW], f32, tag="mlo")
                nc.sync.dma_start(out=mlo[:], in_=macc[64:128, :])
                nc.vector.tensor_add(msg[:, t * TW:(t + 1) * TW], macc[0:64, :], mlo[:])

            # zero pad atoms, then BN2 stats
            if "2" not in PH:
                nc.vector.memset(msg[:, 0:NPAD], 1.0)
            nc.vector.memset(msg[:, NLOC:NPAD], 0.0)
            st2 = apool.tile([F, 2], f32)
            nc.vector.tensor_reduce(st2[:, 0:1], msg[:], axis=mybir.AxisListType.X,
                                    op=mybir.AluOpType.add)
            ysq2 = apool.tile([F, NT], f32)
            for t in range(NT):
                sq2t = pool.tile([F, TW], f32, tag="sq2t")
                nc.scalar.activation(out=sq2t[:], in_=msg[:, t * TW:(t + 1) * TW],
                                     func=AF.Square, accum_out=ysq2[:, t:t + 1])
            nc.vector.tensor_reduce(st2[:, 1:2], ysq2[:], axis=mybir.AxisListType.X,
                                    op=mybir.AluOpType.add)
            nc.sync.dma_start(out=st2_loc[:], in_=st2[:])
            nc.gpsimd.collective_compute(
                "AllReduce", mybir.AluOpType.add, replica_groups=groups,
                ins=[st2_loc[:]], outs=[st2_sh[:]])
            st2g = apool.tile([F, 2], f32)
            nc.sync.dma_start(out=st2g[:], in_=st2_sh[:])

            mu2 = apool.tile([F, 1], f32)
            nc.vector.tensor_scalar_mul(mu2[:], st2g[:, 0:1], 1.0 / BN2_CNT)
            var2 = apool.tile([F, 1], f32)
            nc.vector.tensor_scalar_mul(var2[:], st2g[:, 1:2], 1.0 / BN2_CNT)
            msq2 = apool.tile([F, 1], f32)
            nc.vector.tensor_mul(msq2[:], mu2[:], mu2[:])
            nc.vector.tensor_sub(var2[:], var2[:], msq2[:])
            nc.vector.tensor_scalar_add(var2[:], var2[:], EPS)
            lnv2 = apool.tile([F, 1], f32)
            nc.scalar.activation(out=lnv2[:], in_=var2[:], func=AF.Ln)
            nc.vector.tensor_scalar_mul(lnv2[:], lnv2[:], -0.5)
            inv2 = apool.tile([F, 1], f32)
            nc.scalar.activation(out=inv2[:], in_=lnv2[:], func=AF.Exp)
            s2 = apool.tile([F, 1], f32)
            nc.vector.tensor_mul(s2[:], g2_sb[:], inv2[:])
            t2 = apool.tile([F, 1], f32)
            nc.vector.tensor_mul(t2[:], mu2[:], s2[:])
            nc.vector.tensor_sub(t2[:], b2_sb[:], t2[:])

            # --- pass 3: out = atom_fea + softplus(bn2(msg)) ---
            for t in range(NT if "3" in PH else 1):
                aff = pool.tile([64, TW], f32, tag="aff")
                nc.vector.tensor_scalar(out=aff[:], in0=msg[:, t * TW:(t + 1) * TW],
                                        scalar1=s2[:, 0:1], scalar2=t2[:, 0:1],
                                        op0=mybir.AluOpType.mult,
                                        op1=mybir.AluOpType.add)
                ex3 = pool.tile([64, TW], f32, tag="ex3")
                nc.scalar.activation(out=ex3[:], in_=aff[:], func=AF.Exp)
                sp2a = pool.tile([64, TW], f32, tag="sp2a")
                nc.scalar.activation(out=sp2a[:], in_=ex3[:], func=AF.Ln, bias=1.0)
                sp2 = pool.tile([64, TW], f32, tag="sp2")
                nc.vector.tensor_copy(out=sp2[:], in_=sp2a[:])
                for a in range(4):
                    tp = pp.tile([128, 64], f32, tag="tp")
                    nc.tensor.transpose(tp[:], sp2[:, a * 128:(a + 1) * 128], id_sb[:])
                    arow = pool.tile([128, F], f32, tag="arow")
                    n0 = t * TW + a * 128
                    nc.scalar.dma_start(out=arow[:], in_=aself[n0:n0 + 128, :])
                    osb = pool.tile([128, F], f32, tag="osb")
                    nc.vector.tensor_add(osb[:], tp[:], arow[:])
                    nc.sync.dma_start(out=out[n0:n0 + 128, :], in_=osb[:])

    _split_wait_lists(nc)
    return nc


_NC_CACHE = None


def _get_nc():
    global _NC_CACHE
    if _NC_CACHE is None:
        _NC_CACHE = _build()
    return _NC_CACHE


def _prep_core(af_pad, nf, ni, c):
    """Build per-core input map. af_pad: [N+1, 64] f32 global table, row N zero."""
    lo = c * NLOC
    ashard = np.zeros((NPAD, F), np.float32)
    ashard[:NLOC] = af_pad[lo:lo + NLOC]
    nfs = np.zeros((NPAD, M, F), np.float32)
    nfs[:NLOC] = nf[lo:lo + NLOC]
    nis = np.full((NPAD, M), N, np.int64)  # pad edges -> zero row
    nis[:NLOC] = ni[lo:lo + NLOC]

    gath = af_pad[nis.reshape(-1)].reshape(NPAD, M, F)          # [NPAD, M, F]
    # [NT, M, 64, TW] feature-major
    nbrT = nfs.reshape(NT, TW, M, F).transpose(0, 2, 3, 1)
    gathT = gath.reshape(NT, TW, M, F).transpose(0, 2, 3, 1)
    mvT = np.concatenate([nbrT, gathT], axis=2).astype(ml_dtypes.bfloat16)
    atomT = ashard.reshape(NT, TW, F).transpose(0, 2, 1).astype(ml_dtypes.bfloat16)
    return {
        "mvT": np.ascontiguousarray(mvT),
        "atomT": np.ascontiguousarray(atomT),
        "aself": ashard,
    }


def kernel(atom_fea, nbr_fea, nbr_idx, W_full, b_full,
           bn1_gamma, bn1_beta, bn2_gamma, bn2_beta):
    atom_fea = np.asarray(atom_fea, np.float32)
    nbr_fea = np.asarray(nbr_fea, np.float32)
    nbr_idx = np.asarray(nbr_idx)
    W_full = np.asarray(W_full, np.float32)
    bn1_gamma = np.asarray(bn1_gamma, np.float32)
    bn1_beta = np.asarray(bn1_beta, np.float32)
    bn2_gamma = np.asarray(bn2_gamma, np.float32)
    bn2_beta = np.asarray(bn2_beta, np.float32)

    af_pad = np.zeros((N + 1, F), np.float32)
    af_pad[:N] = atom_fea

    # W_full rows: [0:64]=self, [64:128]=gathered nbr atom, [128:192]=edge fea
    w_en = np.ascontiguousarray(np.concatenate([W_full[128:192], W_full[64:128]], axis=0))
    w_self = np.ascontiguousarray(W_full[0:64])

    shared = {
        "w_en": w_en,
        "w_self": w_self,
        "gam1": bn1_gamma.reshape(OUT, 1).copy(),
        "bet1": bn1_beta.reshape(OUT, 1).copy(),
        "gam2": bn2_gamma.reshape(F, 1).copy(),
        "bet2": bn2_beta.reshape(F, 1).copy(),
        "ident": np.eye(64, dtype=np.float32),
    }
    in_maps = []
    for c in range(NCORES):
        m = _prep_core(af_pad, nbr_fea, nbr_idx, c)
        m.update(shared)
        in_maps.append(m)

    nc = _get_nc()
    res = run_bass_kernel_spmd(nc, in_maps, list(range(NCORES)))
    outs = [res.results[c]["out"][:NLOC] for c in range(NCORES)]
    return np.concatenate(outs, axis=0).astype(np.float32)

